# revision 7
# baseline (speedup 1.0000x reference)
"""Paged GQA decode attention (B=64, HQ=32, HKV=8, D=128) on 8 TRN2 NeuronCores.

Strategy: data-parallel over requests with host-side load balancing.
 - Sort the 64 requests by context_lens descending; slot r of core c gets the
   rank-(r*8+c) request, so every core's slot-r request has a similar length.
 - Each slot is padded to the max-of-8 chunk count (chunks of 128 tokens), so
   all 8 cores execute the SAME static program (SPMD) on different data.
 - Host gathers each request's KV blocks (honoring block_tables) into per-core
   shards: one fused [128, 2048] tile per 128-token chunk holding K transposed
   to [d, l] (no on-chip transposes needed) and V in natural [l, d] layout.
 - Per chunk on device: scores_T[l,hq] = K_h^T.T @ qT (8 matmuls), then
   E = exp(scores + bias) on ScalarE where bias is 0 / -30 per token
   (masks padded/invalid tokens), then PV accumulation acc[hq,d] += E_h.T @ V_h
   (8 col-tiled matmuls into two PSUM banks) and a ones-matmul for the
   softmax denominator. Final division happens on host.
"""

import math
import os
import sys
from contextlib import ExitStack

import numpy as np

for _p in ("/opt/trn_rl_repo", "/root/.axon_site/_ro/trn_rl_repo"):
    if os.path.isdir(_p) and _p not in sys.path:
        sys.path.insert(0, _p)
        break

import concourse.bass as bass
import concourse.tile as tile
from concourse import bacc, mybir
from concourse.bass_utils import run_bass_kernel_spmd

B, HQ, HKV, D, BS, MB = 64, 32, 8, 128, 16, 128
G = HQ // HKV              # 4 query heads per kv head
SCALE = 0.08838834764831845
NCORES = 8
SLOTS = B // NCORES        # 8 request slots per core
CHUNK = 128                # tokens per chunk (= SBUF partitions)
BPC = CHUNK // BS          # blocks per chunk = 8
ROW = HKV * D              # 1024 floats per token row
NEG = -30.0                # additive mask for invalid tokens

last_results = None        # stashed BassKernelResults for test.py

_prog_cache = {}


def _build_program(s_counts):
    f32 = mybir.dt.float32
    C_total = sum(s_counts)
    nc = bacc.Bacc()

    kv_d = nc.declare_dram_parameter("kv", [C_total, CHUNK, 2 * ROW], f32,
                                     isOutput=False)
    qT_d = nc.declare_dram_parameter("qT", [D, SLOTS * HQ], f32, isOutput=False)
    bias_d = nc.declare_dram_parameter("bias", [CHUNK, C_total], f32,
                                       isOutput=False)
    out_d = nc.declare_dram_parameter("out", [SLOTS, HKV, G, D], f32,
                                      isOutput=True)
    den_d = nc.declare_dram_parameter("den", [SLOTS, HQ], f32, isOutput=True)

    EXP = mybir.ActivationFunctionType.Exp

    with tile.TileContext(nc) as tc, ExitStack() as ctx:
        kvpool = ctx.enter_context(tc.tile_pool(name="kv", bufs=4))
        epool = ctx.enter_context(tc.tile_pool(name="e", bufs=3))
        const = ctx.enter_context(tc.tile_pool(name="cst", bufs=1))
        spsum = ctx.enter_context(tc.tile_pool(name="sp", bufs=2, space="PSUM"))
        apsum = ctx.enter_context(tc.tile_pool(name="ac", bufs=2, space="PSUM"))
        dpsum = ctx.enter_context(tc.tile_pool(name="dp", bufs=2, space="PSUM"))

        bias_t = const.tile([CHUNK, C_total], f32)
        nc.sync.dma_start(bias_t[:], bias_d[:])
        q_all = const.tile([D, SLOTS * HQ], f32)
        nc.sync.dma_start(q_all[:], qT_d[:])
        # ones on ScalarE so the denominator matmul's deps stay in the single
        # ACT semaphore domain (PE matmuls support only one sync wait).
        ones = const.tile([CHUNK, 1], f32)
        nc.scalar.activation(ones[:], bias_t[:, 0:1],
                             mybir.ActivationFunctionType.Identity,
                             bias=1.0, scale=0.0)
        # dummy matmul absorbs the q_all DMA wait so the first real matmul
        # only waits on its kv DMA.
        dmy = spsum.tile([1, 1], f32, tag="sco")
        nc.tensor.matmul(dmy[:], q_all[0:1, 0:1], q_all[0:1, 0:1],
                         start=True, stop=True)

        gc = 0
        for r in range(SLOTS):
            S_r = s_counts[r]
            qt = q_all[:, r * HQ:(r + 1) * HQ]
            acc_a = apsum.tile([CHUNK, D], f32, tag="acca")
            acc_b = apsum.tile([CHUNK, D], f32, tag="accb")
            den_p = dpsum.tile([HQ, 1], f32, tag="den")
            for j in range(S_r):
                kv = kvpool.tile([CHUNK, 2 * ROW], f32)
                nc.sync.dma_start(kv[:], kv_d[gc + j])
                kt = kv[:, 0:ROW]        # rows = d, cols = (h, l)
                vt = kv[:, ROW:2 * ROW]  # rows = l, cols = (h, d)

                sco = spsum.tile([CHUNK, HQ], f32, tag="sco")
                for h in range(HKV):
                    nc.tensor.matmul(
                        sco[:, h * G:(h + 1) * G],
                        kt[:, h * D:(h + 1) * D],
                        qt[:, h * G:(h + 1) * G],
                        start=True, stop=True,
                    )
                et = epool.tile([CHUNK, HQ], f32)
                nc.scalar.activation(
                    et[:], sco[:], EXP,
                    bias=bias_t[:, gc + j:gc + j + 1], scale=1.0,
                )
                st, sp = (j == 0), (j == S_r - 1)
                for h in range(HKV):
                    accp = acc_a if h < 4 else acc_b
                    jj = h % 4
                    nc.tensor.matmul(
                        accp[32 * jj:32 * jj + G, :],
                        et[:, h * G:(h + 1) * G],
                        vt[:, h * D:(h + 1) * D],
                        start=st, stop=sp,
                        tile_position=(0, 32 * jj),
                    )
                nc.tensor.matmul(den_p[:], et[:], ones[:], start=st, stop=sp)
            out_sa = epool.tile([CHUNK, D], f32, tag="outa")
            out_sb = epool.tile([CHUNK, D], f32, tag="outb")
            den_s = epool.tile([HQ, 1], f32, tag="dens")
            nc.scalar.copy(out_sa[:], acc_a[:])
            nc.scalar.copy(out_sb[:], acc_b[:])
            nc.scalar.copy(den_s[:], den_p[:])
            for h in range(HKV):
                srcp = out_sa if h < 4 else out_sb
                jj = h % 4
                nc.sync.dma_start(out_d[r, h], srcp[32 * jj:32 * jj + G, :])
            nc.sync.dma_start(den_d[r], den_s[:])
            gc += S_r
    nc.compile()
    return nc


def _get_program(s_counts):
    if s_counts not in _prog_cache:
        _prog_cache[s_counts] = _build_program(s_counts)
    return _prog_cache[s_counts]


def _make_schedule(context_lens):
    L = context_lens.astype(np.int64)
    order = np.argsort(-L, kind="stable")
    s_counts = []
    for r in range(SLOTS):
        grp = order[r * NCORES:(r + 1) * NCORES]
        s_counts.append(max(1, math.ceil(int(L[grp].max()) / CHUNK)))
    return order, tuple(s_counts)


def _build_in_maps(q, k_cache, v_cache, block_tables, L, order, s_counts):
    C_total = sum(s_counts)
    nblocks_total = k_cache.shape[0]
    kf = k_cache.reshape(nblocks_total, BS, ROW)
    vf = v_cache.reshape(nblocks_total, BS, ROW)

    in_maps = []
    core_reqs = []
    for c in range(NCORES):
        kv = np.empty((C_total, CHUNK, 2 * ROW), np.float32)
        biasT = np.empty((C_total, CHUNK), np.float32)
        qT = np.empty((D, SLOTS * HQ), np.float32)
        reqs = []
        gc = 0
        for r in range(SLOTS):
            b = int(order[r * NCORES + c])
            reqs.append(b)
            S_r = s_counts[r]
            blocks = np.clip(block_tables[b, :S_r * BPC].astype(np.int64),
                             0, nblocks_total - 1)
            kreq = kf[blocks].reshape(S_r, CHUNK, HKV, D)
            kv[gc:gc + S_r, :, 0:ROW] = \
                kreq.transpose(0, 3, 2, 1).reshape(S_r, D, ROW)
            kv[gc:gc + S_r, :, ROW:2 * ROW] = vf[blocks].reshape(S_r, CHUNK, ROW)
            tok = np.arange(S_r * CHUNK, dtype=np.int64)
            biasT[gc:gc + S_r] = np.where(tok < int(L[b]), 0.0, NEG) \
                .astype(np.float32).reshape(S_r, CHUNK)
            qT[:, r * HQ:(r + 1) * HQ] = (q[b] * SCALE).T
            gc += S_r
        in_maps.append({
            "kv": kv, "qT": qT,
            "bias": np.ascontiguousarray(biasT.T),
        })
        core_reqs.append(reqs)
    return in_maps, core_reqs


def kernel(q, k_cache, v_cache, block_tables, context_lens):
    global last_results
    q = np.asarray(q, dtype=np.float32)
    k_cache = np.asarray(k_cache, dtype=np.float32)
    v_cache = np.asarray(v_cache, dtype=np.float32)
    block_tables = np.asarray(block_tables, dtype=np.int32)
    context_lens = np.asarray(context_lens, dtype=np.int32)

    L = context_lens.astype(np.int64)
    order, s_counts = _make_schedule(context_lens)
    nc = _get_program(s_counts)
    in_maps, core_reqs = _build_in_maps(
        q, k_cache, v_cache, block_tables, L, order, s_counts)

    res = run_bass_kernel_spmd(
        nc, in_maps, list(range(NCORES)),
        trace=bool(os.environ.get("KBASS_TRACE")),
    )
    last_results = res

    out = np.empty((B, HQ, D), np.float32)
    for c in range(NCORES):
        acc = res.results[c]["out"].reshape(SLOTS, HQ, D)
        den = res.results[c]["den"].reshape(SLOTS, HQ)
        o = acc / den[:, :, None]
        for r, b in enumerate(core_reqs[c]):
            out[b] = o[r]
    return out
